# revision 26
# baseline (speedup 1.0000x reference)
"""Feedforward SNN (Linear -> LIF) x2 kernel for Trainium2, 8-core data parallel.

Per-core plan (B sharded 8 ways, BL=32 samples/core):
  - Host pre-transposes operands once (cheap numpy) and Dekker-splits EVERY
    matmul operand into fp16 hi+lo pairs (pre-scaled by powers of 2 to dodge
    fp16 subnormals). fp16 runs the PE at 1.0 cycles/row (vs fp32's 4.0) and
    its 11-bit mantissa is held exactly by the PE's internal FP22 format, so
    a 2-term split carries ~22 significant bits -- fp32-grade for this model
    (validated: end-to-end error below the fp32 BLAS-reorder noise floor).
  - Layer-1 currents for ALL timesteps: Cur1[h1, (t,b)] = W1 @ x^T as THREE
    fp16 matmuls (W1h*xh + W1l*xh + W1h*xl; the dropped W1l*xl term is
    ~2^-22 relative). 3 passes at 1x rate vs fp32's 4x-slow single pass.
  - LIF-1 scan over t on [128, HC1*32] tiles (partition = h1 % 128, free =
    (h1chunk, b)); fused scalar_tensor_tensor DVE ops, 3/step.
  - Spikes are {0,1} == exact in fp16; layer-2 currents are 2x fp16 matmuls
    (W2h + W2l = 22-bit W2) accumulated in fp32 PSUM.
  - The pre-scales are undone for free in the PSUM->SBUF bias-add
    activation (out = psum*scale + bias).
  - LIF-2 scan likewise (2 DVE ops/step; spikes materialized only at t=63).
  - Software-pipelined: mm1(nb+1) is emitted before mm2(nb) so the PE fills
    the scan1(nb) latency; PE phases are chained with order-only deps.
"""

import os
import sys

import numpy as np

for _p in ("/opt/trn_rl_repo", "/root/.axon_site/_ro/trn_rl_repo"):
    if os.path.isdir(_p) and _p not in sys.path:
        sys.path.insert(0, _p)

import ml_dtypes  # noqa: E402

import concourse.bass as bass  # noqa: E402
import concourse.mybir as mybir  # noqa: E402
import concourse.tile as tile  # noqa: E402
from concourse import bacc  # noqa: E402
from concourse.bass_utils import run_bass_kernel_spmd  # noqa: E402
from concourse.masks import make_identity  # noqa: E402
from concourse.tile_rust import add_dep_helper  # noqa: E402

F32 = mybir.dt.float32
F32R = mybir.dt.float32r
BF16 = mybir.dt.bfloat16
F16 = mybir.dt.float16
F8 = mybir.dt.float8e4
ALU = mybir.AluOpType
AF = mybir.ActivationFunctionType

BETA = 0.9
THR = 1.0

# fp16 Dekker-split pre-scales (powers of 2; exact in fp32) and the evac
# scales that undo them during the PSUM->SBUF bias-add.
S_X = 16.0
S_W1 = 256.0
S_W2 = 256.0
SC1 = 1.0 / (S_X * S_W1)
SC2 = 1.0 / S_W2

# W2 is expressed as NT fp8(e4m3) terms, each holding the next ~4 bits of
# the residual, consumed by DoubleRow matmuls (0.5 cyc/row).  Term k is
# stored scaled by W2_TS[k] so it sits in e4m3's normal range; the matching
# spike tensor is valued {0, 1/W2_TS[k]} so every product lands at the SAME
# scale and all terms share one PSUM accumulation.  1/TS must be exactly
# representable in e4m3 (>= 2^-9, its smallest subnormal), capping TS at
# 2^9 -- term 3 therefore captures slightly less than 4 extra bits.
W2_TS = [1.0, 16.0, 256.0, 512.0]
NT = len(W2_TS)

B_FULL, T_FULL, D_FULL, H1_FULL, H2_FULL = 256, 64, 1024, 2048, 2048
N_CORES = 8
BL = B_FULL // N_CORES  # 32


def build_snn(T=T_FULL, D=D_FULL, H1=H1_FULL, H2=H2_FULL, T_NB=16):
    """Build the single-core Bass program (identical across the 8 cores)."""
    P = 128
    KC1 = D // P
    HC1 = H1 // P
    HC2 = H2 // P
    NNB = T // T_NB
    SUB = min(4, T_NB)
    NSUB = T_NB // SUB
    MCQ = min(4, HC2)
    HCQ = min(4, HC1)
    NB32 = T_NB * 32          # matmul free dim per t-block

    assert T % T_NB == 0 and T_NB % SUB == 0
    assert HC2 % MCQ == 0 and HC1 % HCQ == 0

    nc = bacc.Bacc("TRN2", target_bir_lowering=False, debug=False)

    xt_d = nc.dram_tensor("xThl", [D, 2, T * BL], F16, kind="ExternalInput")
    w1t_d = nc.dram_tensor("W1Thl", [D, 2, H1], F16, kind="ExternalInput")
    b1_d = nc.dram_tensor("b1", [H1], F32, kind="ExternalInput")
    # fp8 W2 terms in DoubleRow layout: row (q*128+p) holds h1-pair
    # (2q*128+p, (2q+1)*128+p) along the j axis.
    w2t_d = nc.dram_tensor("W2dr", [H1 // 2, NT, 2, H2], F8,
                           kind="ExternalInput")
    b2_d = nc.dram_tensor("b2", [H2], F32, kind="ExternalInput")

    spk2_d = nc.dram_tensor("spk2", [BL, H2], F32, kind="ExternalOutput")
    mem1_d = nc.dram_tensor("mem1", [BL, H1], F32, kind="ExternalOutput")
    mem2_d = nc.dram_tensor("mem2", [BL, H2], F32, kind="ExternalOutput")

    with tile.TileContext(nc) as tc:
        from contextlib import ExitStack
        ctx = ExitStack()
        with ctx:
            const = ctx.enter_context(tc.tile_pool(name="const", bufs=1))
            xtp = ctx.enter_context(tc.tile_pool(name="xtp", bufs=2))
            w1tp = ctx.enter_context(tc.tile_pool(name="w1tp", bufs=5))
            w2tp = ctx.enter_context(tc.tile_pool(name="w2tp", bufs=4))
            curp = ctx.enter_context(tc.tile_pool(name="curp", bufs=6))
            spk1p = ctx.enter_context(tc.tile_pool(name="spk1p", bufs=1))
            statep = ctx.enter_context(tc.tile_pool(name="statep", bufs=2))
            negzp = ctx.enter_context(tc.tile_pool(name="negzp", bufs=1))
            outp = ctx.enter_context(tc.tile_pool(name="outp", bufs=4))
            tpsum = ctx.enter_context(
                tc.tile_pool(name="tpsum", bufs=2, space="PSUM"))
            mpsum = ctx.enter_context(
                tc.tile_pool(name="mpsum", bufs=6, space="PSUM"))

            ident = const.tile([P, P], F32, name="ident")
            make_identity(nc, ident)

            # PE phase chaining (order-only deps): keeps fp32-mm, bf16-mm
            # and transpose phases from interleaving in the PE stream.
            pe_phases = []

            class _Ph:
                def __init__(self):
                    self.insts = []

                def add(self, bi):
                    self.insts.append(bi.ins)

            b1s = const.tile([P, HC1], F32, name="b1s")
            nc.gpsimd.dma_start(
                b1s[:], b1_d.ap().rearrange("(c p) -> p c", p=P))
            b2s = const.tile([P, HC2], F32, name="b2s")
            nc.gpsimd.dma_start(
                b2s[:], b2_d.ap().rearrange("(c p) -> p c", p=P))

            # ---------------- PE warmup (HAM ramp) --------------------------
            wub = const.tile([P, 256], BF16, name="wub")
            nc.vector.memset(wub[:], 0.0)
            wuw = const.tile([P, P], BF16, name="wuw")
            nc.vector.memset(wuw[:], 0.0)
            ph = _Ph()
            pe_phases.append(ph)
            wups = mpsum.tile([P, NB32], F32, tag="mm", name="wups")
            for i in range(20):
                ph.add(nc.tensor.matmul(wups[:, 0:256], wuw[:], wub[:],
                                        start=(i == 0), stop=(i == 19)))

            # ---------------- initial LIF state ----------------------------
            mem1_cur = statep.tile([P, HC1, 32], F32, tag="mem1",
                                   name="mem1_0")
            nc.vector.memset(mem1_cur[:], 0.0)
            mem2_cur = statep.tile([P, HC2, 32], F32, tag="mem2",
                                   name="mem2_0")
            nc.vector.memset(mem2_cur[:], 0.0)
            spk2_fin = const.tile([P, HC2, 32], F32, name="spk2_fin")

            # ---------------- outputs helper --------------------------------
            def emit_out(state, nch, out_d):
                ph = _Ph()
                pe_phases.append(ph)
                for hc in range(nch):
                    ps = tpsum.tile([32, P], F32, tag="tp", name="ops")
                    ph.add(nc.tensor.transpose(ps[:], state[:, hc, :],
                                               ident[:]))
                    sb = outp.tile([32, P], F32, tag="osb", name="osb")
                    nc.scalar.activation(sb[:], ps[:], AF.Copy)
                    nc.sync.dma_start(
                        out_d.ap()[:, hc * P:(hc + 1) * P], sb[:])

            # ---------------- per-block emitters ----------------------------
            def x_and_mm1(nb):
                """xT load + matmul1 for block nb -> cur1_subs"""
                ph = _Ph()
                pe_phases.append(ph)
                t0 = nb * T_NB
                xt = xtp.tile([P, KC1, 2, NB32], F16, tag="xt", name="xt")
                # block 0 is latency-critical at startup: spread its 8 chunk
                # loads over two otherwise-idle queues (weights own sync/
                # scalar); later blocks prefetch leisurely on gpsimd.
                for kc in range(KC1):
                    xq = (nc.gpsimd if kc % 2 == 0 else nc.scalar) \
                        if nb == 0 else nc.gpsimd
                    xq.dma_start(
                        xt[:, kc, :, :],
                        xt_d.ap()[kc * P:(kc + 1) * P, :,
                                  t0 * 32:(t0 + T_NB) * 32])

                cur1_subs = [curp.tile([P, SUB, HC1, 32], F32, tag="cur1",
                                       bufs=4, name="cur1")
                             for _ in range(NSUB)]
                for hq in range(HC1 // HCQ):
                    pss = [mpsum.tile([P, NB32], F32, tag="mm", name="mm1ps")
                           for _ in range(HCQ)]
                    for kc in range(KC1):
                        w1tt = w1tp.tile([P, 2, HCQ * P], F16, tag="w1t",
                                         name="w1tt")
                        dq = nc.sync if kc % 2 == 0 else nc.scalar
                        dq.dma_start(
                            w1tt[:],
                            w1t_d.ap()[kc * P:(kc + 1) * P, :,
                                       hq * HCQ * P:(hq + 1) * HCQ * P])
                        rhs_h = xt[:, kc, 0, :]
                        rhs_l = xt[:, kc, 1, :]
                        for i in range(HCQ):
                            # W1h*xh + W1h*xl + W1l*xh (~22-bit effective);
                            # consecutive same-stationary passes share LDW.
                            ph.add(nc.tensor.matmul(
                                pss[i][:], w1tt[:, 0, i * P:(i + 1) * P],
                                rhs_h, start=(kc == 0), stop=False))
                            ph.add(nc.tensor.matmul(
                                pss[i][:], w1tt[:, 0, i * P:(i + 1) * P],
                                rhs_l, start=False, stop=False))
                            ph.add(nc.tensor.matmul(
                                pss[i][:], w1tt[:, 1, i * P:(i + 1) * P],
                                rhs_h, start=False, stop=(kc == KC1 - 1)))
                    for s in range(NSUB):
                        for i in range(HCQ):
                            hc = hq * HCQ + i
                            psv = pss[i].rearrange("p (t b) -> p t b", b=32)
                            nc.scalar.activation(
                                cur1_subs[s][:, :, hc, :],
                                psv[:, s * SUB:(s + 1) * SUB, :],
                                AF.Identity, bias=b1s[:, hc:hc + 1],
                                scale=SC1)
                return cur1_subs

            # ---------------- scan emitters ---------------------------------
            def scan1(cur1_subs):
                """LIF-1 scan (T_NB steps) -> NT fp8 spike tiles laid out
                [(kc-pair q, t, b, j)] with the DoubleRow pair j BYTE-ADJACENT
                (the XBUS can only feed 2 fp8/cycle from one 16-bit read);
                term k valued {0, 1/W2_TS[k]}."""
                nonlocal mem1_cur
                KP = HC1 // 2
                spk8 = [spk1p.tile([P, KP, NB32, 2], F8, tag=f"spk8_{k}",
                                   bufs=2, name=f"spk8_{k}")
                        for k in range(NT)]
                for tr in range(T_NB):
                    cur_t = cur1_subs[tr // SUB][:, tr % SUB]  # [P, HC1, 32]
                    negz = negzp.tile([P, HC1, 32], F32, tag="negz",
                                      name="negz")
                    nc.vector.scalar_tensor_tensor(
                        negz[:], mem1_cur[:], THR, cur_t,
                        ALU.is_gt, ALU.subtract)
                    mem1_new = statep.tile([P, HC1, 32], F32, tag="mem1",
                                           name="mem1")
                    nc.vector.scalar_tensor_tensor(
                        mem1_new[:], mem1_cur[:], BETA, negz[:],
                        ALU.mult, ALU.subtract)
                    mem1_cur = mem1_new
                    # spike of step t thresholds the POST-update membrane;
                    # iterate (q, j, b) to match mem1's (hc=2q+j, b) order
                    for k in range(NT):
                        sl = spk8[k][:, :, tr * 32:(tr + 1) * 32, :] \
                            .rearrange("p q b j -> p q j b")
                        if W2_TS[k] == 1.0:
                            nc.vector.tensor_scalar(
                                sl, mem1_cur[:], THR, None, ALU.is_gt)
                        else:
                            nc.vector.tensor_scalar(
                                sl, mem1_cur[:], THR, 1.0 / W2_TS[k],
                                ALU.is_gt, ALU.mult)
                return spk8

            def mm2(spk8):
                """cur2[(t,mc,b)] = W2 @ spk^T + b2 (NT fp8 DoubleRow
                terms, all accumulating into one PSUM group)."""
                ph = _Ph()
                pe_phases.append(ph)
                cur2_subs = [curp.tile([P, SUB, HC2, 32], F32, tag="cur2",
                                       bufs=4, name="cur2")
                             for _ in range(NSUB)]
                KP = HC1 // 2
                for mq in range(HC2 // MCQ):
                    pss = [mpsum.tile([P, NB32], F32, tag="mm", name="mm2ps")
                           for _ in range(MCQ)]
                    for q in range(KP):
                        wt = w2tp.tile([P, NT, 2, MCQ * P], F8, tag="w2t",
                                       bufs=4, name="w2t")
                        dq = nc.sync if q % 2 == 0 else nc.scalar
                        dq.dma_start(
                            wt[:],
                            w2t_d.ap()[q * P:(q + 1) * P, :, :,
                                       mq * MCQ * P:(mq + 1) * MCQ * P])
                        for k in range(NT):
                            rhs = spk8[k][:, q].rearrange("p n j -> p j n")
                            for i in range(MCQ):
                                ph.add(nc.tensor.matmul(
                                    pss[i][:],
                                    wt[:, k, :, i * P:(i + 1) * P], rhs,
                                    start=(q == 0 and k == 0),
                                    stop=(q == KP - 1 and k == NT - 1),
                                    perf_mode=mybir.MatmulPerfMode.DoubleRow))
                    # sub-major evac order so scan2 step 0's inputs (s=0 of
                    # every mc) complete as early as possible
                    for s in range(NSUB):
                        for i in range(MCQ):
                            mc = mq * MCQ + i
                            psv = pss[i].rearrange("p (t b) -> p t b", b=32)
                            nc.scalar.activation(
                                cur2_subs[s][:, :, mc, :],
                                psv[:, s * SUB:(s + 1) * SUB, :],
                                AF.Identity, bias=b2s[:, mc:mc + 1],
                                scale=SC2)
                return cur2_subs

            def scan2(cur2_subs, nb):
                nonlocal mem2_cur
                t0 = nb * T_NB
                for tr in range(T_NB):
                    t = t0 + tr
                    cur_t = cur2_subs[tr // SUB][:, tr % SUB]
                    negz = negzp.tile([P, HC2, 32], F32, tag="negz",
                                      name="negz")
                    nc.vector.scalar_tensor_tensor(
                        negz[:], mem2_cur[:], THR, cur_t,
                        ALU.is_gt, ALU.subtract)
                    mem2_new = statep.tile([P, HC2, 32], F32, tag="mem2",
                                           name="mem2")
                    nc.vector.scalar_tensor_tensor(
                        mem2_new[:], mem2_cur[:], BETA, negz[:],
                        ALU.mult, ALU.subtract)
                    mem2_cur = mem2_new
                    if t == T - 1:
                        nc.vector.tensor_scalar(
                            spk2_fin[:], mem2_cur[:], THR, None, ALU.is_gt)

            # ---------------- main t-block pipeline -------------------------
            # Software pipelining, two levels:
            #  - PE stream: mm1(nb+1) is emitted BEFORE mm2(nb) so the PE
            #    fills the scan1 latency instead of stalling on spk1.
            #  - DVE stream: scan1(nb+1) is emitted BEFORE scan2(nb) so the
            #    (FIFO) vector engine runs scan1(nb+1) during mm2(nb) instead
            #    of queuing it behind scan2(nb), which can only start once
            #    mm2(nb) is nearly done. This keeps spk1(nb+1) ready the
            #    moment mm2(nb) retires -- critical for the last block, where
            #    no mm1(nb+1) exists to hide the wait.
            cur1_next = x_and_mm1(0)
            spk1_next = scan1(cur1_next)
            for nb in range(NNB):
                spk1_cur = spk1_next
                if nb + 1 < NNB:
                    cur1_next = x_and_mm1(nb + 1)
                if nb == NNB - 1:
                    emit_out(mem1_cur, HC1, mem1_d)
                cur2_subs = mm2(spk1_cur)
                if nb + 1 < NNB:
                    spk1_next = scan1(cur1_next)
                scan2(cur2_subs, nb)

            # ---------------- remaining outputs -----------------------------
            emit_out(mem2_cur, HC2, mem2_d)
            emit_out(spk2_fin, HC2, spk2_d)

            # chain consecutive PE phases: every inst of phase b ordered
            # after the last inst of phase a (order-only deps)
            for a, b in zip(pe_phases, pe_phases[1:]):
                if a.insts and b.insts:
                    for bi in b.insts:
                        add_dep_helper(bi, a.insts[-1], sync=False,
                                       reason="PE phase ordering")

    nc.compile()
    return nc


_NC_CACHE = {}


def _get_nc():
    if "full" not in _NC_CACHE:
        _NC_CACHE["full"] = build_snn()
    return _NC_CACHE["full"]


def _dekker_f16(a):
    """Split fp32 array into fp16 hi+lo terms stacked on axis 1."""
    hi = a.astype(np.float16)
    lo = (a - hi.astype(np.float32)).astype(np.float16)
    return np.ascontiguousarray(np.stack([hi, lo], axis=1))


def prep_inputs(x, W1, b1, W2, b2):
    """Host-side prep: shard x over cores (transposed to [d, (t,b)]) and
    Dekker-split x, W1, W2 into pre-scaled fp16 hi+lo pairs."""
    x = np.asarray(x, np.float32)
    W1 = np.asarray(W1, np.float32)
    b1 = np.ascontiguousarray(np.asarray(b1, np.float32))
    W2 = np.asarray(W2, np.float32)
    b2 = np.ascontiguousarray(np.asarray(b2, np.float32))
    B, T, D = x.shape

    W1Thl = _dekker_f16(W1.T * np.float32(S_W1))        # [D, 2, H1]

    # W2 -> NT fp8 terms, residual-coded, then packed into the DoubleRow
    # pair layout [H1//2, NT, 2, H2] (row q*128+p holds h1=(2q+j)*128+p).
    H1 = W2.shape[0]
    W2s = np.ascontiguousarray(W2.T) * np.float32(S_W2)  # [H1, H2]
    terms = []
    r = W2s
    for ts in W2_TS:
        t8 = (r * np.float32(ts)).astype(ml_dtypes.float8_e4m3)
        terms.append(t8)
        r = r - t8.astype(np.float32) / np.float32(ts)
    ta = np.stack(terms, 0).reshape(NT, H1 // 256, 2, 128, -1)
    W2dr = np.ascontiguousarray(
        ta.transpose(1, 3, 0, 2, 4).reshape(H1 // 2, NT, 2, -1))

    bl = B // N_CORES
    in_maps = []
    for c in range(N_CORES):
        xc = x[c * bl:(c + 1) * bl]                     # [bl, T, D]
        xT = xc.transpose(2, 1, 0).reshape(D, T * bl)   # [d, (t,b)] t-major
        xThl = _dekker_f16(xT * np.float32(S_X))        # [D, 2, (t,b)]
        in_maps.append({
            "xThl": xThl, "W1Thl": W1Thl, "b1": b1, "W2dr": W2dr,
            "b2": b2,
        })
    return in_maps


def kernel(x, W1, b1, W2, b2):
    """Full-input entry point: shards B across 8 NeuronCores, returns full
    (spk2, mem1, mem2) exactly like reference()."""
    nc = _get_nc()
    in_maps = prep_inputs(x, W1, b1, W2, b2)
    res = run_bass_kernel_spmd(nc, in_maps, core_ids=list(range(N_CORES)))
    spk2 = np.concatenate([res.results[c]["spk2"] for c in range(N_CORES)], 0)
    mem1 = np.concatenate([res.results[c]["mem1"] for c in range(N_CORES)], 0)
    mem2 = np.concatenate([res.results[c]["mem2"] for c in range(N_CORES)], 0)
    return spk2, mem1, mem2

